# revision 22
# baseline (speedup 1.0000x reference)
"""nn_CompoundPoissonQKV kernel for Trainium2 (8 NeuronCores).

Self-contained: hardcodes B=4, N=4096, D=256, TOPK=49.
Sharding: core c -> sample c//2, row-half c%2 (data parallel over batch,
sequence parallel over rows). Per-core program is identical; all per-core
variation is input data. GCN halves are exchanged with a pairwise AllGather.

v2 design notes (engine-balanced, cost-model-driven):
  - scores linearized: softmax arg = cos/256 in [-1/256,1/256], so
    exp(x) == 1+x below bf16 ulp. scoresT = 1 + cos/256 folded into the
    transpose-psum eviction; expsum comes from an appended ones column in
    the phase-B matmul (column 256 of gcnf). No Exp => single ACT table.
  - support = X @ (W_V @ gcn_w * SCALE) fused on-chip (VT eliminated).
  - adj = dots * (dots >= thr) via tensor_scalar is_ge (DVE 4x bf16) +
    tensor_tensor mult (DVE 2x); both row-major, then PE-transposed.
  - wide [128,1024] PSUM tiles amortize eviction init overhead.
  - engine assignment tuned to measured cost model: ACT evicts dots psum
    (f32) + adjT; Pool evicts scoresT (scale+bias); DVE owns topk + LN.
  - phase A software-pipelined (dots matmul of tile t+1 issued before the
    dependent tail of tile t) to keep PE p-state warm.
"""
import sys
sys.path.insert(0, '/opt/trn_rl_repo')
import numpy as np
import concourse.bass as bass
import concourse.bacc as bacc
import concourse.mybir as mybir
import concourse.tile as tile
from concourse import masks

F32 = mybir.dt.float32
BF16 = mybir.dt.bfloat16
AF = mybir.ActivationFunctionType
ALU = mybir.AluOpType

P = 128
D = 256
N = 4096
NH = 2048
T = NH // P          # 16 row tiles per core
C = N // P           # 32 col chunks
KTOP = 49
SCALE = 1.0 / 16.0
CEXP = SCALE * SCALE  # softmax linearization slope (1/256)
EPS = 1e-5
NEG = -3.0e38
DSTRIDE = D + 1      # gcnf/support column stride (ones column at offset D)


def build(n_cores=8):
    stage = "full"
    nc = bacc.Bacc("TRN2", target_bir_lowering=False, debug=False,
                   num_devices=n_cores)

    xf_d = nc.dram_tensor("Xf", [N, D], F32, kind="ExternalInput")
    xh_d = nc.dram_tensor("Xh", [NH, D], F32, kind="ExternalInput")
    wq_d = nc.dram_tensor("wq", [D, D], F32, kind="ExternalInput")
    wk_d = nc.dram_tensor("wk", [D, D], F32, kind="ExternalInput")
    wv_d = nc.dram_tensor("wv", [D, D], F32, kind="ExternalInput")
    gw_d = nc.dram_tensor("gw", [D, D], F32, kind="ExternalInput")
    lng_d = nc.dram_tensor("lng", [1, D], F32, kind="ExternalInput")
    lnb_d = nc.dram_tensor("lnb", [1, D], F32, kind="ExternalInput")
    o_d = nc.dram_tensor("o", [NH, D], F32, kind="ExternalOutput")

    with tile.TileContext(nc) as tc:
        _body(nc, tc, xf_d, xh_d, wq_d, wk_d, wv_d, gw_d, lng_d, lnb_d, o_d,
              stage=stage)
    nc.compile()
    return nc


def _body(nc, tc, xf_d, xh_d, wq_d, wk_d, wv_d, gw_d, lng_d, lnb_d, o_d,
          stage="full"):
    from contextlib import ExitStack
    es = ExitStack()

    consts = es.enter_context(tc.tile_pool(name="consts", bufs=1))
    big = es.enter_context(tc.tile_pool(name="big", bufs=1))
    work = es.enter_context(tc.tile_pool(name="work", bufs=3))
    small = es.enter_context(tc.tile_pool(name="small", bufs=3))
    ps_dots = es.enter_context(tc.tile_pool(name="ps_dots", bufs=2, space="PSUM"))
    ps_tr = es.enter_context(tc.tile_pool(name="ps_tr", bufs=2, space="PSUM"))
    ps_mm = es.enter_context(tc.tile_pool(name="ps_mm", bufs=2, space="PSUM"))
    dram = es.enter_context(tc.tile_pool(name="dram", bufs=1, space="DRAM"))

    # ---------------- constants ----------------
    ident = consts.tile([P, P], BF16)
    masks.make_identity(nc, ident[:])

    def load_w_bf16(w_d, nm):
        out = []
        for kc in range(2):
            wf = small.tile([P, D], F32, tag="wstage", name=f"wf{nm}{kc}")
            nc.sync.dma_start(wf[:], w_d[kc * P:(kc + 1) * P, :])
            wb = consts.tile([P, D], BF16, name=f"{nm}{kc}")
            nc.scalar.activation(wb[:], wf[:], AF.Copy)
            out.append(wb)
        return out

    WQ = load_w_bf16(wq_d, "wqb")
    WK = load_w_bf16(wk_d, "wkb")
    WV = load_w_bf16(wv_d, "wvb")
    GW = load_w_bf16(gw_d, "gwb")

    # LN params as bf16 (used post-normalization on bf16 y1)
    G = consts.tile([P, D], BF16)
    Bb = consts.tile([P, D], BF16)
    g1 = small.tile([1, D], F32, tag="lnstage", name="g1")
    nc.sync.dma_start(g1[:], lng_d[:])
    gb = small.tile([1, D], BF16, tag="lnstage2", name="gb")
    nc.scalar.activation(gb[:], g1[:], AF.Copy)
    nc.gpsimd.partition_broadcast(G[:], gb[:])
    b1 = small.tile([1, D], F32, tag="lnstage", name="b1")
    nc.sync.dma_start(b1[:], lnb_d[:])
    bb = small.tile([1, D], BF16, tag="lnstage2", name="bb")
    nc.scalar.activation(bb[:], b1[:], AF.Copy)
    nc.gpsimd.partition_broadcast(Bb[:], bb[:])

    # Wf = (W_V @ gw) * SCALE, bf16 chunks [128, 256]
    # Wf[k,d] = sum_j Wv[k,j] gw[j,d]; lhsT = WvT chunk [j,k], rhs = GW [j,d]
    WvT = []
    for kc in range(2):
        tp = ps_tr.tile([P, 2 * P], BF16, tag="trps", name=f"wvtp{kc}")
        for jc in range(2):
            nc.tensor.transpose(tp[:, jc * P:(jc + 1) * P],
                                WV[jc][:, kc * P:(kc + 1) * P], ident[:])
        wt = consts.tile([P, 2 * P], BF16, name=f"wvT{kc}")
        nc.vector.tensor_copy(wt[:], tp[:])
        WvT.append(wt)
    WF = []
    for kc in range(2):
        fp = ps_mm.tile([P, DSTRIDE], F32, tag="mmps", name=f"wfps{kc}")
        for jc in range(2):
            nc.tensor.matmul(fp[:, 0:D], WvT[jc][:, kc * P:(kc + 1) * P],
                             GW[jc][:], start=(jc == 0), stop=(jc == 1))
        wfb = consts.tile([P, D], BF16, name=f"wfb{kc}")
        nc.scalar.activation(wfb[:], fp[:, 0:D], AF.Copy, scale=SCALE)
        WF.append(wfb)

    # ---------------- persistent big tensors ----------------
    KT = [big.tile([P, N], BF16, name=f"KT{dc}") for dc in range(2)]
    QT = [big.tile([P, NH], BF16, name=f"QT{dc}") for dc in range(2)]
    # support/gcnf share a slot: [p, c*257+d] = sup[c*128+p, d]; col 256 spare
    support = big.tile([P, C * DSTRIDE], BF16, tag="support")

    # DRAM scratch
    scoresd = dram.tile([NH, N], BF16)
    gcnhalf = dram.tile([NH, D], BF16)
    gcnfull = dram.tile([N, D], BF16)

    # ---------------- preprocessing ----------------
    # XT aliases phase-A work tags (dead before first phase-A write).
    with tc.tile_pool(name="prep_stage", bufs=4) as pstage:
        XT = [work.tile([P, N], BF16, name=f"XT{dc}", tag=wtag)
              for dc, wtag in ((0, "adj_b"), (1, "adjT"))]
        XhT = [work.tile([P, NH], BF16, name=f"XhT{dc}", tag=wtag)
               for dc, wtag in ((0, "scoresT"), (1, "mask_b"))]

        def x_chunk(src_d, dst, tps, cn, ci):
            # load + convert one [128,256] X chunk, transpose into tps
            xf = pstage.tile([P, D], F32, tag="xstage")
            nc.sync.dma_start(xf[:], src_d[cn * P:(cn + 1) * P, :])
            xb = pstage.tile([P, D], BF16, tag="xbstage")
            if ci % 2 == 0:
                nc.vector.tensor_copy(xb[:], xf[:])
            else:
                nc.gpsimd.tensor_copy(xb[:], xf[:])
            for dc in range(2):
                nc.tensor.transpose(tps[dc][:, ci * P:(ci + 1) * P],
                                    xb[:, dc * P:(dc + 1) * P], ident[:])

        def norm_chunk(xt_src, w, tps, cn, ci):
            # rows = l2norm(x @ w) for one [128] row chunk; transpose into tps
            rp = ps_mm.tile([P, DSTRIDE], F32, tag="mmps")
            for kc in range(2):
                nc.tensor.matmul(rp[:, 0:D], xt_src[kc][:, cn * P:(cn + 1) * P],
                                 w[kc][:], start=(kc == 0), stop=(kc == 1))
            sq = pstage.tile([P, D], F32, tag="sqscr")
            ss = small.tile([P, 1], F32, tag="ss")
            nc.scalar.activation(sq[:], rp[:, 0:D], AF.Square, accum_out=ss[:])
            nrm = small.tile([P, 1], F32, tag="nrm")
            nc.scalar.activation(nrm[:], ss[:], AF.Sqrt)
            nc.vector.tensor_scalar_max(nrm[:], nrm[:], 1e-12)
            inv = small.tile([P, 1], F32, tag="inv")
            nc.vector.reciprocal(inv[:], nrm[:])
            rn = pstage.tile([P, D], BF16, tag="rnorm")
            if cn % 2 == 0:
                nc.scalar.activation(rn[:], rp[:, 0:D], AF.Copy, scale=inv[:])
            else:
                nc.vector.tensor_scalar(rn[:], rp[:, 0:D], inv[:], None,
                                        op0=ALU.mult)
            for dc in range(2):
                nc.tensor.transpose(tps[dc][:, ci * P:(ci + 1) * P],
                                    rn[:, dc * P:(dc + 1) * P], ident[:])

        def sup_chunk(cn):
            sp = ps_mm.tile([P, DSTRIDE], F32, tag="mmps")
            for kc in range(2):
                nc.tensor.matmul(sp[:, 0:D], XT[kc][:, cn * P:(cn + 1) * P],
                                 WF[kc][:], start=(kc == 0), stop=(kc == 1))
            if cn % 2 == 0:
                nc.vector.tensor_copy(
                    support[:, cn * DSTRIDE:cn * DSTRIDE + D], sp[:, 0:D])
            else:
                nc.scalar.activation(
                    support[:, cn * DSTRIDE:cn * DSTRIDE + D], sp[:, 0:D],
                    AF.Copy)

        GS = 4

        def flush(tps, dst, cg):
            for dc in range(2):
                nc.vector.tensor_copy(dst[dc][:, cg * GS * P:(cg + 1) * GS * P],
                                      tps[dc][:])

        def new_tps(nm):
            return [ps_tr.tile([P, GS * P], BF16, tag="trps", name=f"{nm}{dc}")
                    for dc in range(2)]

        # chunk-pipelined: X chunks feed K-norm + support one group behind
        xtps = ktps = None
        for cg in range(C // GS + 1):
            if cg < C // GS:
                ntps = new_tps("xt")
                for ci in range(GS):
                    x_chunk(xf_d, XT, ntps, cg * GS + ci, ci)
            if cg > 0:
                flush(xtps, XT, cg - 1)
                ktps_n = new_tps("kt")
                for ci in range(GS):
                    cn = (cg - 1) * GS + ci
                    norm_chunk(XT, WK, ktps_n, cn, ci)
                    sup_chunk(cn)
                if cg > 1:
                    flush(ktps, KT, cg - 2)
                ktps = ktps_n
            xtps = ntps if cg < C // GS else xtps
        flush(ktps, KT, C // GS - 1)

        # Xh -> XhT -> QT (small; same pipelined pattern)
        htps = qtps = None
        for cg in range(T // GS + 1):
            if cg < T // GS:
                ntps = new_tps("ht")
                for ci in range(GS):
                    x_chunk(xh_d, XhT, ntps, cg * GS + ci, ci)
            if cg > 0:
                flush(htps, XhT, cg - 1)
                qtps_n = new_tps("qt")
                for ci in range(GS):
                    norm_chunk(XhT, WQ, qtps_n, (cg - 1) * GS + ci, ci)
                if cg > 1:
                    flush(qtps, QT, cg - 2)
                qtps = qtps_n
            htps = ntps if cg < T // GS else htps
        flush(qtps, QT, T // GS - 1)

    # ---------------- phase A ----------------
    if stage == "prep":
        of0 = small.tile([P, D], F32, tag="of")
        nc.vector.tensor_copy(of0[:], G[:])
        nc.sync.dma_start(o_d[0:P, :], of0[:])
        es.close()
        return

    nt_a = 1 if stage == "a1" else T

    def emit_s1(t):
        # PE: dots psum; ACT: evict to dots_b bf16
        dots_b = work.tile([P, N], BF16, tag="dots_b")
        for cc in range(4):
            dp = ps_dots.tile([P, 1024], F32, tag="dps")
            for hh in range(2):
                sl = slice(cc * 1024 + hh * 512, cc * 1024 + (hh + 1) * 512)
                for kc in range(2):
                    nc.tensor.matmul(dp[:, hh * 512:(hh + 1) * 512],
                                     QT[kc][:, t * P:(t + 1) * P],
                                     KT[kc][:, sl],
                                     start=(kc == 0), stop=(kc == 1))
            nc.scalar.activation(dots_b[:, cc * 1024:(cc + 1) * 1024], dp[:],
                                 AF.Copy)
        return dots_b

    def emit_s2(t, dots_b):
        # DVE: topk threshold chain + adjacency (row-major)
        cand = work.tile([P, P], F32, tag="cand")
        for cidx in range(16):
            nc.vector.max(out=cand[:, cidx * 8:(cidx + 1) * 8],
                          in_=dots_b[:, cidx * 2 * P:(cidx + 1) * 2 * P])
        m0 = small.tile([P, 8], F32, tag="m0")
        nc.vector.max(out=m0[:], in_=cand[:])
        nc.vector.match_replace(out=cand[:], in_to_replace=m0[:],
                                in_values=cand[:], imm_value=NEG)
        for r in range(1, 6):
            mr = small.tile([P, 8], F32, tag="mr")
            nc.vector.max(out=mr[:], in_=cand[:])
            nc.vector.match_replace(out=cand[:], in_to_replace=mr[:],
                                    in_values=cand[:], imm_value=NEG)
        m6 = small.tile([P, 8], F32, tag="m6")
        nc.vector.max(out=m6[:], in_=cand[:])
        thr = small.tile([P, 1], F32, tag="thr")
        nc.vector.tensor_tensor(thr[:], m6[:, 0:1], m6[:, 1:2], op=ALU.add)
        nc.vector.tensor_scalar_mul(thr[:], thr[:], 0.5)

        mask_b = work.tile([P, N], BF16, tag="mask_b")
        nc.gpsimd.tensor_scalar(mask_b[:, 0:2048], dots_b[:, 0:2048], thr[:],
                                None, op0=ALU.is_ge)
        nc.vector.tensor_scalar(mask_b[:, 2048:N], dots_b[:, 2048:N], thr[:],
                                None, op0=ALU.is_ge)
        adj_b = work.tile([P, N], BF16, tag="adj_b")
        nc.vector.tensor_tensor(adj_b[:, 0:2048], dots_b[:, 0:2048],
                                mask_b[:, 0:2048], op=ALU.mult)
        nc.gpsimd.tensor_tensor(adj_b[:, 2048:N], dots_b[:, 2048:N],
                                mask_b[:, 2048:N], op=ALU.mult)
        return adj_b

    def emit_s3(t, dots_b, adj_b):
        # transposes + evictions + gcn + layernorm/relu
        adjT = work.tile([P, N], BF16, tag="adjT")
        scoresT = work.tile([P, N], BF16, tag="scoresT")
        for cg in range(4):
            tpd = ps_tr.tile([P, 1024], BF16, tag="trps", name="tpd")
            for ci in range(8):
                cn = cg * 8 + ci
                nc.tensor.transpose(tpd[:, ci * P:(ci + 1) * P],
                                    dots_b[:, cn * P:(cn + 1) * P], ident[:])
            if cg == 0:
                nc.vector.tensor_scalar(scoresT[:, cg * 1024:(cg + 1) * 1024],
                                        tpd[:], CEXP, 1.0, op0=ALU.mult,
                                        op1=ALU.add)
            else:
                nc.scalar.activation(scoresT[:, cg * 1024:(cg + 1) * 1024],
                                     tpd[:], AF.Copy, scale=CEXP, bias=1.0)
            tpa = ps_tr.tile([P, 1024], BF16, tag="trps", name="tpa")
            for ci in range(8):
                cn = cg * 8 + ci
                nc.tensor.transpose(tpa[:, ci * P:(ci + 1) * P],
                                    adj_b[:, cn * P:(cn + 1) * P], ident[:])
            if cg % 2 == 0:
                nc.scalar.activation(adjT[:, cg * 1024:(cg + 1) * 1024],
                                     tpa[:], AF.Copy)
            else:
                nc.vector.tensor_copy(adjT[:, cg * 1024:(cg + 1) * 1024],
                                      tpa[:])
        nc.sync.dma_start(scoresd[t * P:(t + 1) * P, :], scoresT[:])

        gp = ps_mm.tile([P, DSTRIDE], F32, tag="mmps")
        for cn in range(C):
            nc.tensor.matmul(gp[:, 0:D], adjT[:, cn * P:(cn + 1) * P],
                             support[:, cn * DSTRIDE:cn * DSTRIDE + D],
                             start=(cn == 0), stop=(cn == C - 1))

        y0 = small.tile([P, D], F32, tag="y0")
        s1 = small.tile([P, 1], F32, tag="s1")
        nc.scalar.activation(y0[:], gp[:, 0:D], AF.Copy, accum_out=s1[:])
        sq = small.tile([P, D], F32, tag="sq")
        s2 = small.tile([P, 1], F32, tag="s2")
        nc.scalar.activation(sq[:], y0[:], AF.Square, accum_out=s2[:])
        mu = small.tile([P, 1], F32, tag="mu")
        nc.vector.tensor_scalar_mul(mu[:], s1[:], 1.0 / D)
        v1 = small.tile([P, 1], F32, tag="v1")
        nc.vector.tensor_scalar(v1[:], s2[:], 1.0 / D, EPS, op0=ALU.mult,
                                op1=ALU.add)
        musq = small.tile([P, 1], F32, tag="musq")
        nc.vector.tensor_tensor(musq[:], mu[:], mu[:], op=ALU.mult)
        nc.vector.tensor_tensor(v1[:], v1[:], musq[:], op=ALU.subtract)
        std = small.tile([P, 1], F32, tag="std")
        nc.scalar.activation(std[:], v1[:], AF.Sqrt)
        istd = small.tile([P, 1], F32, tag="istd")
        nc.vector.reciprocal(istd[:], std[:])
        nb = small.tile([P, 1], F32, tag="nb")
        nc.vector.scalar_tensor_tensor(nb[:], mu[:], -1.0, istd[:],
                                       op0=ALU.mult, op1=ALU.mult)
        y1 = small.tile([P, D], BF16, tag="y1")
        nc.scalar.activation(y1[:], y0[:], AF.Identity, scale=istd[:],
                             bias=nb[:])
        y2 = small.tile([P, D], BF16, tag="y2")
        nc.gpsimd.tensor_tensor(y2[:], y1[:], G[:], op=ALU.mult)
        nc.gpsimd.tensor_tensor(y2[:], y2[:], Bb[:], op=ALU.add)
        gcnb = small.tile([P, D], BF16, tag="gcnb")
        nc.gpsimd.tensor_scalar_max(gcnb[:], y2[:], 0.0)
        nc.sync.dma_start(gcnhalf[t * P:(t + 1) * P, :], gcnb[:])

    # 3-deep software pipeline, stalest stage emitted first per iteration
    st = {}
    for t in range(nt_a + 2):
        if t - 1 >= 0 and t - 1 < nt_a:
            st[t - 1] = (st[t - 1][0], emit_s2(t - 1, st[t - 1][0]))
        if t - 2 >= 0:
            emit_s3(t - 2, *st.pop(t - 2))
        if t < nt_a:
            st[t] = (emit_s1(t), None)

    # ---------------- all-gather ----------------
    if stage in ("a1", "a"):
        es.close()
        return
    if stage == "tl":
        # single-core timing variant: stand in for the all-gather with two
        # local copies of the half (wrong numerics, right data volume)
        nc.sync.dma_start(gcnfull[0:NH, :], gcnhalf[:, :])
        nc.sync.dma_start(gcnfull[NH:N, :], gcnhalf[:, :])
    else:
        nc.gpsimd.collective_compute(
            "AllGather", ALU.bypass,
            replica_groups=[[0, 1], [2, 3], [4, 5], [6, 7]],
            ins=[gcnhalf.opt()], outs=[gcnfull.opt()],
        )

    if stage == "cc":
        es.close()
        return
    # gcnf reuses the support slot; ones column at offset 256 of each chunk
    gcnf = big.tile([P, C * DSTRIDE], BF16, tag="support")
    nc.sync.dma_start(
        gcnf[:].rearrange("p (c d) -> p c d", c=C)[:, :, 0:D],
        gcnfull[:].rearrange("(c p) d -> p c d", p=P))
    nc.vector.memset(
        gcnf[:].rearrange("p (c d) -> p c d", c=C)[:, :, D:DSTRIDE], 1.0)

    # ---------------- phase B ----------------
    for t in range(T):
        scT = work.tile([P, N], BF16, tag="dots_b")  # reuse dots_b slots
        nc.sync.dma_start(scT[:], scoresd[t * P:(t + 1) * P, :])
        ap = ps_mm.tile([P, DSTRIDE], F32, tag="mmps")
        for cn in range(C):
            nc.tensor.matmul(ap[:], scT[:, cn * P:(cn + 1) * P],
                             gcnf[:, cn * DSTRIDE:(cn + 1) * DSTRIDE],
                             start=(cn == 0), stop=(cn == C - 1))
        rz = small.tile([P, 1], F32, tag="rz")
        nc.vector.reciprocal(rz[:], ap[:, D:DSTRIDE])
        of = small.tile([P, D], F32, tag="of")
        nc.scalar.activation(of[:], ap[:, 0:D], AF.Copy, scale=rz[:])
        nc.sync.dma_start(o_d[t * P:(t + 1) * P, :], of[:])

    es.close()


# ---------------- host side ----------------
class Runner:
    def __init__(self, n_cores=8):
        self.n_cores = n_cores
        self.nc = build(n_cores)
        self._fn = None

    def _prepare(self):
        import jax
        import jax.numpy as jnp
        from jax.sharding import Mesh, PartitionSpec
        from jax.experimental.shard_map import shard_map
        from concourse import bass2jax
        from concourse.bass2jax import _bass_exec_p, partition_id_tensor
        bass2jax.install_neuronx_cc_hook()
        nc = self.nc
        partition_name = (nc.partition_id_tensor.name
                          if nc.partition_id_tensor else None)
        in_names, out_names, out_avals = [], [], []
        for alloc in nc.m.functions[0].allocations:
            if not isinstance(alloc, mybir.MemoryLocationSet):
                continue
            name = alloc.memorylocations[0].name
            if alloc.kind == "ExternalInput":
                if name != partition_name:
                    in_names.append(name)
            elif alloc.kind == "ExternalOutput":
                out_names.append(name)
                out_avals.append(jax.core.ShapedArray(
                    tuple(alloc.tensor_shape), mybir.dt.np(alloc.dtype)))
        self.in_names = in_names
        self.out_names = out_names
        self.out_avals = out_avals
        n_params = len(in_names)
        all_names = in_names + out_names
        if partition_name is not None:
            all_names = all_names + [partition_name]

        def _b(*args):
            operands = list(args)
            if partition_name is not None:
                operands.append(partition_id_tensor())
            outs = _bass_exec_p.bind(
                *operands,
                out_avals=tuple(out_avals),
                in_names=tuple(all_names),
                out_names=tuple(out_names),
                lowering_input_output_aliases=(),
                sim_require_finite=False,
                sim_require_nnan=False,
                nc=nc,
            )
            return tuple(outs)

        devices = jax.devices()[:self.n_cores]
        self.mesh = Mesh(np.asarray(devices), ("core",))
        specs = (PartitionSpec("core"),) * (n_params + len(out_names))
        out_specs = (PartitionSpec("core"),) * len(out_names)
        self._fn = jax.jit(shard_map(_b, mesh=self.mesh, in_specs=specs,
                                     out_specs=out_specs, check_rep=False))

    def device_args(self, in_maps):
        import jax
        if self._fn is None:
            self._prepare()
        concat = [np.concatenate([np.ascontiguousarray(in_maps[c][n])
                                  for c in range(self.n_cores)], axis=0)
                  for n in self.in_names]
        zeros = [np.zeros((self.n_cores * av.shape[0], *av.shape[1:]), av.dtype)
                 for av in self.out_avals]
        return [jax.device_put(a) for a in concat + zeros]

    def execute(self, dev_args):
        return self._fn(*dev_args)

    def run(self, in_maps):
        args = self.device_args(in_maps)
        outs = self.execute(args)
        res = []
        for c in range(self.n_cores):
            d = {}
            for i, nm in enumerate(self.out_names):
                av = self.out_avals[i]
                d[nm] = np.asarray(outs[i]).reshape(self.n_cores, *av.shape)[c]
            res.append(d)
        return res


_RUNNER = None


def kernel(X, W_Q, W_K, W_V, gcn_w, ln_g, ln_b):
    global _RUNNER
    if _RUNNER is None:
        _RUNNER = Runner(8)
    in_maps = make_in_maps(X, W_Q, W_K, W_V, gcn_w, ln_g, ln_b)
    try:
        res = _RUNNER.run(in_maps)
    except Exception:
        # rare transient NRT exec failure: rebuild the executable once
        _RUNNER = Runner(8)
        res = _RUNNER.run(in_maps)
    return assemble(res)


def make_in_maps(X, W_Q, W_K, W_V, gcn_w, ln_g, ln_b):
    X = np.asarray(X, np.float32)
    in_maps = []
    for c in range(8):
        b, h = c // 2, c % 2
        in_maps.append({
            "Xf": X[b],
            "Xh": X[b, h * NH:(h + 1) * NH],
            "wq": np.asarray(W_Q, np.float32),
            "wk": np.asarray(W_K, np.float32),
            "wv": np.asarray(W_V, np.float32),
            "gw": np.asarray(gcn_w, np.float32)[b],
            "lng": np.asarray(ln_g, np.float32)[b:b + 1],
            "lnb": np.asarray(ln_b, np.float32)[b:b + 1],
        })
    return in_maps


def assemble(res):
    out = np.zeros((4, N, D), np.float32)
    for c in range(8):
        b, h = c // 2, c % 2
        out[b, h * NH:(h + 1) * NH] = res[c]["o"]
    return out
